# revision 11
# baseline (speedup 1.0000x reference)
"""Block-sparse matmul + bias + relu on 8 Trainium2 NeuronCores.

Strategy (data-parallel over batch):
  - Shard x along batch: 8 cores x 512 rows. w_blocks/bias replicated.
  - Per core, compute out^T with the PE in 32x32 tiling mode:
      * x^T resident in SBUF as [128, 32, 512]: input block i lives at
        partitions 32*(i%4) .. 32*(i%4)+31, free tile i//4.
      * each nonzero block (i,j) is one matmul: lhsT = w_block [K=32, M=32],
        rhs = x^T block i [32, 512], tile_position=(32*(i%4), 32*(j%4)).
  - Four PSUM banks per output quad, one per row group (different row tiles
    must not accumulate into the same PSUM bank - verified to hang on HW).
  - Per quad combine: ACT folds bias while evacuating bank1 + copies bank3,
    DVE adds bank0/bank2 and the cross-pair sum, GPSIMD only does the final
    relu via tensor_scalar (its tensor-tensor ops are ~2x slower than DVE).
  - Host: transpose/cast prep (bf16 feeds the PE; fp32 accumulate in PSUM).
"""

import os

import numpy as np
import ml_dtypes

import concourse.bass as bass
import concourse.tile as tile
from concourse import mybir
from concourse.bass_utils import run_bass_kernel_spmd

LAST_RESULTS = None  # test-only: BassKernelResults of the last run

BS = 32
KB = 128
NB = 128
BATCH = 4096
NCORES = 8
BC = BATCH // NCORES          # 512 batch rows per core
NQ = NB // 4                  # 32 quads of output block-cols
SEP = 6                       # min issue distance between different row tiles
                              # accumulating the same (bank, strip) region
IN_DT = mybir.dt.bfloat16
IN_NP = ml_dtypes.bfloat16
OUT_DT = mybir.dt.float32
OUT_NP = np.float32

_CACHE = {}


def _build_schedule(row_idx, col_idx):
    """Two-bank schedule. Returns (sched, S, slot_of, dummy_slots) where
    sched[q] is a list of (r, c, t, slot, start, stop); bank is r % 2.
    Greedy emission alternates bank parity (helps LDWEIGHTS pull-ahead and
    keeps both banks' write streams interleaved) while enforcing SEP between
    different row groups hitting the same (bank, strip)."""
    nnz = len(row_idx)
    sched = []
    slot_ctr = [0, 0, 0, 0]           # per row-group strip in the w image
    slot_of = {}
    # one shared zero-weight slot per row group for dummies
    zero_slot = [None, None, None, None]

    def get_zero_slot(r):
        if zero_slot[r] is None:
            zero_slot[r] = slot_ctr[r]
            slot_ctr[r] += 1
        return zero_slot[r]

    fifos_all = [[[[] for _ in range(4)] for _ in range(4)] for _ in range(NQ)]
    for n in range(nnz):
        i = int(row_idx[n]); j = int(col_idx[n])
        fifos_all[j // 4][i % 4][j % 4].append(n)

    for q in range(NQ):
        fifos = fifos_all[q]          # fifos[r][c]
        # every (bank=r, c) region must be written at least once so the
        # combine reads defined PSUM
        for c in range(4):
            for r in range(4):
                if not fifos[r][c]:
                    fifos[r][c].append(None)

        emitted = []                  # [r, c, t, slot, start, stop]
        last_writer = {}              # (bank, c) -> (rowgroup, idx)
        last_r = None
        remaining = sum(len(fifos[r][c]) for r in range(4) for c in range(4))
        while remaining:
            best = None
            best_score = None
            for r in range(4):
                for c in range(4):
                    if not fifos[r][c]:
                        continue
                    score = 0
                    if last_r is not None and (r % 2) != (last_r % 2):
                        score += 100
                    if last_r is not None and r != last_r:
                        score += 10
                    score += len(fifos[r][c])
                    if best_score is None or score > best_score:
                        best_score = score
                        best = (r, c)
            assert best is not None
            r, c = best
            n = fifos[r][c].pop(0)
            remaining -= 1
            if n is None:
                slot = get_zero_slot(r)
                t = 0
            else:
                slot = slot_ctr[r]; slot_ctr[r] += 1
                slot_of[n] = slot
                t = int(row_idx[n]) // 4
            emitted.append([r, c, t, slot, False, False])
            last_r = r

        # start/stop per (bank=r, strip) accumulation region
        first_seen = set()
        for e in emitted:
            key = (e[0], e[1])
            if key not in first_seen:
                e[4] = True
                first_seen.add(key)
        last_idx = {}
        for k, e in enumerate(emitted):
            last_idx[(e[0], e[1])] = k
        for k in last_idx.values():
            emitted[k][5] = True
        sched.append([tuple(e) for e in emitted])

    S = max(slot_ctr)
    dummy_slots = [(r, s) for r, s in enumerate(zero_slot) if s is not None]
    return sched, S, slot_of, dummy_slots


_MULTIWAIT_OK = {"InstDMACopy", "InstUnconditionalBranch",
                 "InstConditionalBranch"}


def _legalize_waits(nc):
    """Engine ISA structs carry a single sync-wait slot; Tile can emit more.
    Offload excess waits onto same-engine NoOps inserted just before the
    instruction (per-engine stream order is the block list order)."""
    ctr = 0
    for f in nc.m.functions:
        for blk in f.blocks:
            out = []
            for inst in blk.instructions:
                si = inst.sync_info
                if (si is not None and si.on_wait and len(si.on_wait) > 1
                        and type(inst).__name__ == "InstDMACopy"):
                    # HWDGE lane sems are monotonic add-only counters; a
                    # DMA's wait on its own completion lane orders it against
                    # unrelated prior DMAs on that lane and is droppable.
                    own = {u.ant_name for u in (si.on_update or [])}
                    keep = [w for w in si.on_wait if w.ant_name not in own]
                    if len(keep) > 1:
                        raise RuntimeError(
                            f"DMA {inst.name} still has waits {keep}")
                    inst.sync_info = mybir.SyncInfo(on_wait=keep,
                                                    on_update=si.on_update)
                    out.append(inst)
                    continue
                if (si is not None and si.on_wait and len(si.on_wait) > 1
                        and type(inst).__name__ not in _MULTIWAIT_OK):
                    waits = list(si.on_wait)
                    for w in waits[:-1]:
                        nop = mybir.InstNoOp(name=f"waitnop-{ctr}")
                        ctr += 1
                        nop.engine = inst.engine
                        nop.sync_info = mybir.SyncInfo(on_wait=[w], on_update=[])
                        out.append(nop)
                    inst.sync_info = mybir.SyncInfo(on_wait=[waits[-1]],
                                                    on_update=si.on_update)
                out.append(inst)
            blk.instructions[:] = out


def _build_program(sched, S, repeat=1, loop_n=0, dyn_loop=False):
    nc = bass.Bass("TRN2", target_bir_lowering=False, debug=False,
                   num_devices=NCORES)
    x_d = nc.dram_tensor("xt", [128, 32 * BC], IN_DT, kind="ExternalInput").ap()
    w_d = nc.dram_tensor("wim", [128, S * 32], IN_DT, kind="ExternalInput").ap()
    b_d = nc.dram_tensor("bias", [128, 32], mybir.dt.float32,
                         kind="ExternalInput").ap()
    o_d = nc.dram_tensor("outT", [NQ, 128, BC], OUT_DT, kind="ExternalOutput").ap()
    ln_d = None
    if dyn_loop:
        ln_d = nc.dram_tensor("loopn", [1, 1], mybir.dt.uint32,
                              kind="ExternalInput").ap()

    import contextlib

    with tile.TileContext(nc) as tc:
        if dyn_loop:
            tmp = nc.alloc_registers("loopn_tmp", mybir.ALL_ENGINES)
            nc.regs_load(tmp, ln_d[0:1, 0:1])
            loop_end = nc.snap(tmp, donate=True, min_val=0, max_val=1 << 20)
            loop_cm = tc.For_i(0, loop_end, 1)
        elif loop_n:
            loop_cm = tc.For_i(0, loop_n, 1)
        else:
            loop_cm = contextlib.nullcontext()
        with tc.tile_pool(name="const", bufs=1) as cpool, \
             tc.tile_pool(name="work", bufs=3) as wpool, \
             tc.tile_pool(name="psum", bufs=2, space="PSUM") as ppool, \
             loop_cm:
            xt = cpool.tile([128, 32 * BC], IN_DT)
            wt = cpool.tile([128, S * 32], IN_DT)
            bt = cpool.tile([128, 32], mybir.dt.float32)
            nc.sync.dma_start(bt[:], b_d[:])
            # x: chunked DMA (16 x 1MB)
            xch = (32 * BC) // 16
            for k in range(16):
                nc.sync.dma_start(xt[:, k * xch:(k + 1) * xch],
                                  x_d[:, k * xch:(k + 1) * xch])
            # w: chunked DMA in slot order so early quads unblock early
            wch = 8
            wstep = -(-S // wch) * 32
            for k in range(wch):
                lo = k * wstep
                hi = min(S * 32, lo + wstep)
                if lo >= hi:
                    continue
                nc.sync.dma_start(wt[:, lo:hi], w_d[:, lo:hi])

            for rep in range(repeat):
              for q in range(NQ):
                acc = [ppool.tile([128, BC], mybir.dt.float32, tag=f"acc{b}",
                                  name=f"acc{b}_q{q}_p{rep}")
                       for b in range(4)]
                for (r, c, t, slot, start, stop) in sched[q]:
                    nc.tensor.matmul(
                        out=acc[r][32 * c:32 * c + 32, :],
                        lhsT=wt[32 * r:32 * r + 32,
                                slot * 32:(slot + 1) * 32],
                        rhs=xt[32 * r:32 * r + 32, t * BC:(t + 1) * BC],
                        start=start, stop=stop,
                        tile_position=(32 * r, 32 * c),
                        skip_group_check=True,
                    )
                c1 = wpool.tile([128, BC], mybir.dt.float32, tag="c1")
                c3 = wpool.tile([128, BC], mybir.dt.float32, tag="c3")
                s1 = wpool.tile([128, BC], mybir.dt.float32, tag="s1")
                s2 = wpool.tile([128, BC], mybir.dt.float32, tag="s2")
                s3 = wpool.tile([128, BC], mybir.dt.float32, tag="s3")
                ot = wpool.tile([128, BC], OUT_DT, tag="ot")
                # ACT: evacuate banks 1/3 (bias folded into bank1)
                nc.scalar.activation(c1[:], acc[1][:],
                                     mybir.ActivationFunctionType.Identity,
                                     bias=bt[:, q:q + 1])
                nc.scalar.copy(c3[:], acc[3][:])
                # DVE: fold banks 0/2 and the cross sum
                nc.vector.tensor_add(s1[:], acc[0][:], c1[:])
                nc.vector.tensor_add(s2[:], acc[2][:], c3[:])
                nc.vector.tensor_add(s3[:], s1[:], s2[:])
                # GPSIMD: final relu only (tensor_scalar max)
                nc.gpsimd.tensor_scalar_max(ot[:], s3[:], 0.0)
                nc.sync.dma_start(o_d[q], ot[:])
    _legalize_waits(nc)
    return nc


def _prep_inputs(x, w_blocks, bias, row_idx, col_idx, slot_of, dummy_slots, S):
    nnz = len(row_idx)
    # x^T images per core: [128, 32, BC] -> block i at partitions 32*(i%4),
    # free tile i//4.  x[b, 32*(4t+r)+p] -> xt[32r+p, t, b]
    xb = x.astype(IN_NP).reshape(BATCH, 32, 4, 32)        # b, t, r, p
    xt_all = np.ascontiguousarray(xb.transpose(2, 3, 1, 0))  # r, p, t, b
    xt_all = xt_all.reshape(128, 32, BATCH)
    xts = [np.ascontiguousarray(xt_all[:, :, c * BC:(c + 1) * BC]
                                ).reshape(128, 32 * BC) for c in range(NCORES)]
    # w image [128, S*32]
    wim = np.zeros((128, S * 32), dtype=IN_NP)
    wb = w_blocks.astype(IN_NP)
    for n in range(nnz):
        r = int(row_idx[n]) % 4
        s = slot_of[n]
        wim[32 * r:32 * r + 32, 32 * s:32 * s + 32] = wb[n]
    # dummy slots already zero
    bim = np.ascontiguousarray(
        bias.astype(np.float32).reshape(32, 4, 32).transpose(1, 2, 0)
    ).reshape(128, 32)
    return xts, wim, bim


def kernel(x, w_blocks, bias, row_idx, col_idx):
    repeat = int(os.environ.get("BASS_KERNEL_REPEAT", "1"))
    key = (row_idx.tobytes(), col_idx.tobytes(), repeat)
    if key not in _CACHE:
        sched, S, slot_of, dummy_slots = _build_schedule(row_idx, col_idx)
        nc = _build_program(sched, S, repeat=repeat)
        _CACHE[key] = (nc, S, (slot_of, dummy_slots))
    nc, S, aux = _CACHE[key]

    slot_of, dummy_slots = aux
    xts, wim, bim = _prep_inputs(x, w_blocks, bias, row_idx, col_idx,
                                 slot_of, dummy_slots, S)
    in_maps = [{"xt": xts[c], "wim": wim, "bias": bim} for c in range(NCORES)]
    trace = bool(os.environ.get("BASS_KERNEL_TRACE"))
    res = run_bass_kernel_spmd(nc, in_maps, list(range(NCORES)), trace=trace)
    global LAST_RESULTS
    LAST_RESULTS = res

    out = np.empty((BATCH, NB * BS), dtype=np.float32)
    for c in range(NCORES):
        outT = res.results[c]["outT"].reshape(NB * BS, BC)
        out[c * BC:(c + 1) * BC, :] = outT.T.astype(np.float32)
    return out


# revision 17
# speedup vs baseline: 2.7304x; 2.7304x over previous
"""Block-sparse matmul + bias + relu on 8 Trainium2 NeuronCores.

Strategy (data-parallel over batch):
  - Shard x along batch: 8 cores x 512 rows. w_blocks/bias replicated.
  - PE in 32x32 tiling mode: each nonzero block (i,j) is one matmul
    (lhsT = w block [32,32], rhs = x^T block i [32,512]) at
    tile_position (32*(i%4), 32*(j%4)). 16 tiles run concurrently; the
    per-block LDWEIGHTS stream (~26.7ns per 32-col load) is the floor.
  - Four PSUM banks per output quad, one per row group (different row
    tiles must not accumulate into the same PSUM bank - hangs on HW).
  - Per quad combine (engine-balanced, GPSIMD unused - its tensor ops
    measure ~5us each here):
      ACT:  c1 = acc1 + bias (PSUM->SBUF f32), c3 = copy(acc3)
      DVE:  s1 = acc0 + c1 -> bf16, s2 = acc2 + c3 -> bf16,
            s3 = s1 + s2 (bf16 2x mode)
      ACT:  ot = relu(s3) (bf16, emitted one quad late so the ACT queue
            never stalls behind this quad's DVE adds)
  - Output DMA'd as bf16 (halves out traffic); host casts to f32.
  - Input DMAs (x^T image, weight image, bias) hoisted out of the sweep
    loop - they are loop-invariant.
"""

import os

import numpy as np
import ml_dtypes

import concourse.bass as bass
import concourse.tile as tile
from concourse import mybir
from concourse.bass_utils import run_bass_kernel_spmd

LAST_RESULTS = None  # test-only: BassKernelResults of the last run

BS = 32
KB = 128
NB = 128
BATCH = 4096
NCORES = 8
BC = BATCH // NCORES          # 512 batch rows per core
NQ = NB // 4                  # 32 quads of output block-cols
IN_DT = mybir.dt.bfloat16
IN_NP = ml_dtypes.bfloat16
OUT_DT = mybir.dt.bfloat16
OUT_NP = ml_dtypes.bfloat16

_CACHE = {}


def _build_schedule(row_idx, col_idx):
    """Per-quad schedule. Returns (sched, S, slot_of, dummy_slots);
    sched[q] = list of (r, c, t, slot, start, stop), bank = r."""
    nnz = len(row_idx)
    sched = []
    slot_ctr = [0, 0, 0, 0]           # per row-group strip in the w image
    slot_of = {}
    zero_slot = [None, None, None, None]

    def get_zero_slot(r):
        if zero_slot[r] is None:
            zero_slot[r] = slot_ctr[r]
            slot_ctr[r] += 1
        return zero_slot[r]

    fifos_all = [[[[] for _ in range(4)] for _ in range(4)] for _ in range(NQ)]
    for n in range(nnz):
        i = int(row_idx[n]); j = int(col_idx[n])
        fifos_all[j // 4][i % 4][j % 4].append(n)

    for q in range(NQ):
        fifos = fifos_all[q]          # fifos[r][c]
        for c in range(4):
            for r in range(4):
                if not fifos[r][c]:
                    fifos[r][c].append(None)

        emitted = []                  # [r, c, t, slot, start, stop]
        # r cycles fastest: consecutive MMs hit different row groups so the
        # PE can pull the next LDWEIGHTS ahead of in-flight MATMULs.
        maxlen = max(len(fifos[r][c]) for r in range(4) for c in range(4))
        for s in range(maxlen):
            for c in range(4):
                for r in range(4):
                    lst = fifos[r][c]
                    if s >= len(lst):
                        continue
                    n = lst[s]
                    if n is None:
                        slot = get_zero_slot(r)
                        t = 0
                    else:
                        slot = slot_ctr[r]; slot_ctr[r] += 1
                        slot_of[n] = slot
                        t = int(row_idx[n]) // 4
                    emitted.append([r, c, t, slot, False, False])

        # start/stop per (bank=r, strip) accumulation region
        first_seen = set()
        for e in emitted:
            key = (e[0], e[1])
            if key not in first_seen:
                e[4] = True
                first_seen.add(key)
        last_idx = {}
        for k, e in enumerate(emitted):
            last_idx[(e[0], e[1])] = k
        for k in last_idx.values():
            emitted[k][5] = True
        sched.append([tuple(e) for e in emitted])

    S = max(slot_ctr)
    dummy_slots = [(r, s) for r, s in enumerate(zero_slot) if s is not None]
    return sched, S, slot_of, dummy_slots


_MULTIWAIT_OK = {"InstDMACopy", "InstUnconditionalBranch",
                 "InstConditionalBranch"}


def _legalize_waits(nc):
    """Engine ISA structs carry a single sync-wait slot; Tile can emit more.
    Offload excess waits onto same-engine NoOps inserted just before the
    instruction (per-engine stream order is the block list order)."""
    ctr = 0
    for f in nc.m.functions:
        for blk in f.blocks:
            out = []
            for inst in blk.instructions:
                si = inst.sync_info
                if (si is not None and si.on_wait and len(si.on_wait) > 1
                        and type(inst).__name__ == "InstDMACopy"):
                    # HWDGE lane sems are monotonic add-only counters; a
                    # DMA's wait on its own completion lane orders it against
                    # unrelated prior DMAs on that lane and is droppable.
                    own = {u.ant_name for u in (si.on_update or [])}
                    keep = [w for w in si.on_wait if w.ant_name not in own]
                    if len(keep) > 1:
                        raise RuntimeError(
                            f"DMA {inst.name} still has waits {keep}")
                    inst.sync_info = mybir.SyncInfo(on_wait=keep,
                                                    on_update=si.on_update)
                    out.append(inst)
                    continue
                if (si is not None and si.on_wait and len(si.on_wait) > 1
                        and type(inst).__name__ not in _MULTIWAIT_OK):
                    waits = list(si.on_wait)
                    for w in waits[:-1]:
                        nop = mybir.InstNoOp(name=f"waitnop-{ctr}")
                        ctr += 1
                        nop.engine = inst.engine
                        nop.sync_info = mybir.SyncInfo(on_wait=[w], on_update=[])
                        out.append(nop)
                    inst.sync_info = mybir.SyncInfo(on_wait=[waits[-1]],
                                                    on_update=si.on_update)
                out.append(inst)
            blk.instructions[:] = out


def _build_program(sched, S, repeat=1, loop_n=0, dyn_loop=False):
    nc = bass.Bass("TRN2", target_bir_lowering=False, debug=False,
                   num_devices=NCORES)
    x_d = nc.dram_tensor("xt", [128, 32 * BC], IN_DT, kind="ExternalInput").ap()
    w_d = nc.dram_tensor("wim", [128, S * 32], IN_DT, kind="ExternalInput").ap()
    b_d = nc.dram_tensor("bias", [128, 32], mybir.dt.float32,
                         kind="ExternalInput").ap()
    o_d = nc.dram_tensor("outT", [NQ, 128, BC], OUT_DT, kind="ExternalOutput").ap()
    ln_d = None
    if dyn_loop:
        ln_d = nc.dram_tensor("loopn", [1, 1], mybir.dt.uint32,
                              kind="ExternalInput").ap()

    import contextlib

    with tile.TileContext(nc) as tc:
        if dyn_loop:
            tmp = nc.alloc_registers("loopn_tmp", mybir.ALL_ENGINES)
            nc.regs_load(tmp, ln_d[0:1, 0:1])
            loop_end = nc.snap(tmp, donate=True, min_val=0, max_val=1 << 20)
            loop_cm = tc.For_i(0, loop_end, 1)
        elif loop_n:
            loop_cm = tc.For_i(0, loop_n, 1)
        else:
            loop_cm = contextlib.nullcontext()
        with tc.tile_pool(name="const", bufs=1) as cpool, \
             tc.tile_pool(name="work", bufs=4) as wpool, \
             tc.tile_pool(name="psum", bufs=2, space="PSUM") as ppool:
            xt = cpool.tile([128, 32 * BC], IN_DT)
            wt = cpool.tile([128, S * 32], IN_DT)
            bt = cpool.tile([128, 32], mybir.dt.float32)
            # input DMAs hoisted: loop-invariant
            nc.sync.dma_start(bt[:], b_d[:])
            xch = (32 * BC) // 16
            for k in range(16):
                nc.sync.dma_start(xt[:, k * xch:(k + 1) * xch],
                                  x_d[:, k * xch:(k + 1) * xch])
            wch = 8
            wstep = -(-S // wch) * 32
            for k in range(wch):
                lo = k * wstep
                hi = min(S * 32, lo + wstep)
                if lo >= hi:
                    continue
                nc.sync.dma_start(wt[:, lo:hi], w_d[:, lo:hi])

            with loop_cm:
              for rep in range(repeat):
                prev = None             # pending (s3, ot, q) relu+dma
                for q in range(NQ):
                    acc = [ppool.tile([128, BC], mybir.dt.float32,
                                      tag=f"acc{b}", name=f"acc{b}_q{q}_p{rep}")
                           for b in range(4)]
                    for (r, c, t, slot, start, stop) in sched[q]:
                        nc.tensor.matmul(
                            out=acc[r][32 * c:32 * c + 32, :],
                            lhsT=wt[32 * r:32 * r + 32,
                                    slot * 32:(slot + 1) * 32],
                            rhs=xt[32 * r:32 * r + 32, t * BC:(t + 1) * BC],
                            start=start, stop=stop,
                            tile_position=(32 * r, 32 * c),
                            skip_group_check=True,
                        )
                    c1 = wpool.tile([128, BC], mybir.dt.float32, tag="c1")
                    c3 = wpool.tile([128, BC], mybir.dt.float32, tag="c3")
                    s1 = wpool.tile([128, BC], mybir.dt.bfloat16, tag="s1")
                    s2 = wpool.tile([128, BC], mybir.dt.bfloat16, tag="s2")
                    s3 = wpool.tile([128, BC], mybir.dt.bfloat16, tag="s3")
                    # ACT: evacuate banks 1/3 (bias folded into bank1)
                    nc.scalar.activation(c1[:], acc[1][:],
                                         mybir.ActivationFunctionType.Identity,
                                         bias=bt[:, q:q + 1])
                    if prev is not None:
                        ps3, pot, pq = prev
                        nc.scalar.activation(pot[:], ps3[:],
                                             mybir.ActivationFunctionType.Relu)
                        nc.sync.dma_start(o_d[pq], pot[:])
                    nc.scalar.copy(c3[:], acc[3][:])
                    # DVE: fold banks 0/2 (bf16 outputs), then the cross sum
                    nc.vector.tensor_add(s1[:], acc[0][:], c1[:])
                    nc.vector.tensor_add(s2[:], acc[2][:], c3[:])
                    nc.vector.tensor_add(s3[:], s1[:], s2[:])
                    ot = wpool.tile([128, BC], OUT_DT, tag="ot")
                    prev = (s3, ot, q)
                ps3, pot, pq = prev
                nc.scalar.activation(pot[:], ps3[:],
                                     mybir.ActivationFunctionType.Relu)
                nc.sync.dma_start(o_d[pq], pot[:])
    _legalize_waits(nc)
    return nc


def _prep_inputs(x, w_blocks, bias, row_idx, col_idx, slot_of, dummy_slots, S):
    nnz = len(row_idx)
    # x^T images per core: block i at partitions 32*(i%4), free tile i//4.
    xb = x.astype(IN_NP).reshape(BATCH, 32, 4, 32)        # b, t, r, p
    xt_all = np.ascontiguousarray(xb.transpose(2, 3, 1, 0))  # r, p, t, b
    xt_all = xt_all.reshape(128, 32, BATCH)
    xts = [np.ascontiguousarray(xt_all[:, :, c * BC:(c + 1) * BC]
                                ).reshape(128, 32 * BC) for c in range(NCORES)]
    wim = np.zeros((128, S * 32), dtype=IN_NP)
    wb = w_blocks.astype(IN_NP)
    for n in range(nnz):
        r = int(row_idx[n]) % 4
        s = slot_of[n]
        wim[32 * r:32 * r + 32, 32 * s:32 * s + 32] = wb[n]
    bim = np.ascontiguousarray(
        bias.astype(np.float32).reshape(32, 4, 32).transpose(1, 2, 0)
    ).reshape(128, 32)
    return xts, wim, bim


def kernel(x, w_blocks, bias, row_idx, col_idx):
    repeat = int(os.environ.get("BASS_KERNEL_REPEAT", "1"))
    key = (row_idx.tobytes(), col_idx.tobytes(), repeat)
    if key not in _CACHE:
        sched, S, slot_of, dummy_slots = _build_schedule(row_idx, col_idx)
        nc = _build_program(sched, S, repeat=repeat)
        _CACHE[key] = (nc, S, (slot_of, dummy_slots))
    nc, S, aux = _CACHE[key]

    slot_of, dummy_slots = aux
    xts, wim, bim = _prep_inputs(x, w_blocks, bias, row_idx, col_idx,
                                 slot_of, dummy_slots, S)
    in_maps = [{"xt": xts[c], "wim": wim, "bias": bim} for c in range(NCORES)]
    trace = bool(os.environ.get("BASS_KERNEL_TRACE"))
    res = run_bass_kernel_spmd(nc, in_maps, list(range(NCORES)), trace=trace)
    global LAST_RESULTS
    LAST_RESULTS = res

    out = np.empty((BATCH, NB * BS), dtype=np.float32)
    for c in range(NCORES):
        outT = res.results[c]["outT"].reshape(NB * BS, BC)
        out[c * BC:(c + 1) * BC, :] = outT.T.astype(np.float32)
    return out
